# revision 10
# baseline (speedup 1.0000x reference)
"""Trainium2 Bass kernel for the maxtext-style quantized KV-cache update.

Computation (see problem reference):
  1. quantize the new decode-step K/V (per-(b,h) abs-max over D, rint)
  2. scatter-append at ar_cache_index into the stored (S,H,B,D) int8-valued
     cache + per-row scales
  3. return the fully dequantized caches  q * scale / 127.5  for K and V.

Strategy: tensor-parallel over heads — 16 heads -> 2 per NeuronCore, 8 cores.
The kernel is pure HBM-bandwidth-bound, so the cache streams through the
device in float16: the stored cache is int8-valued (rint of small floats,
|q| <= a few hundred), which fp16's 11-bit significand represents EXACTLY,
and the dequantized product only needs ~1e-3 relative accuracy.  This halves
both the read and the write traffic vs f32.  Each core's shard of one cache
is a contiguous (3072 x 1024) stream handled as 3 tiles of (128, 8192) fp16
(2 MB per DMA) with one broadcast multiply per tile.  The updated sequence
row is quantized/dequantized on device in f32 and patched into the output
tile (via the SWDGE queue, off the bulk-load FIFO) before store.
"""

import os
import sys

if "/opt/trn_rl_repo" not in sys.path:
    sys.path.insert(0, "/opt/trn_rl_repo")

# The kernel executes through the axon/neuron PJRT backend; a leftover
# JAX_PLATFORMS=cpu (used for reference-side jax) would hide the NeuronCores.
if "jax" not in sys.modules:
    _jp = os.environ.get("JAX_PLATFORMS")
    if _jp is not None and "axon" not in _jp and "neuron" not in _jp:
        del os.environ["JAX_PLATFORMS"]

import numpy as np

B, H, D = 4, 16, 128
S_AR = 3072
NCORES = 8
HSH = H // NCORES            # heads per core
ROWB = HSH * B * D           # floats per sequence row per core (1024)
F = 8192                     # SBUF tile free dim (fp16: 16 KB / partition)
NT = S_AR * ROWB // (128 * F)  # tiles per cache shard (3)
NBLK = F // D                # scale blocks per partition row (64)
C_DEQ = float(np.float32(1.0 / 127.5))
MAX_INT8 = 127.5
MAGIC = 12582912.0           # 1.5 * 2**23: (x + MAGIC) - MAGIC == rint(x) in f32

TRACE = False                # test harness sets True to capture an NTFF profile
LAST_RESULT = None           # BassKernelResults of the most recent run

_PROG_CACHE = {}


def _build_program(s: int):
    import concourse.bacc as bacc
    import concourse.mybir as mybir
    from concourse.tile import TileContext

    f32 = mybir.dt.float32
    f16 = mybir.dt.float16
    op = mybir.AluOpType

    nc = bacc.Bacc("TRN2", target_bir_lowering=False, debug=False,
                   num_devices=NCORES)

    ck = nc.dram_tensor("ck", [NT, 128, F], f16, kind="ExternalInput")
    cv = nc.dram_tensor("cv", [NT, 128, F], f16, kind="ExternalInput")
    sk = nc.dram_tensor("sk", [128, NT * NBLK], f32, kind="ExternalInput")
    sv = nc.dram_tensor("sv", [128, NT * NBLK], f32, kind="ExternalInput")
    nk = nc.dram_tensor("nk", [HSH * B, D], f32, kind="ExternalInput")
    nv = nc.dram_tensor("nv", [HSH * B, D], f32, kind="ExternalInput")
    ok = nc.dram_tensor("ok", [NT, 128, F], f16, kind="ExternalOutput")
    ov = nc.dram_tensor("ov", [NT, 128, F], f16, kind="ExternalOutput")

    # position of sequence row s inside the (NT, 128, F) tiling
    e0 = s * ROWB
    t_star, rem = divmod(e0, 128 * F)
    p_star, f_star = divmod(rem, F)

    # Queue discipline: both HWDGE rings (sync=SP, scalar=ACT) carry bulk
    # traffic — loads alternate rings, stores take the opposite parity, so
    # each ring sees 3 loads + 3 stores and loads always precede stores in
    # ring-FIFO order (a store blocked on its multiply never delays a
    # load).  Small transfers (rows, scales, patches) ride the scalar ring
    # up front.  No SWDGE (gpsimd) DMA at all: its SBUF descriptor rings
    # sit on AXI ports shared with SDMA engines 7/15, and any SWDGE
    # traffic slows those engines, gating every 16-way-split bulk
    # transfer.  bufs are sized so no queue ever stalls on tile reuse.
    tiles = [(t, nm) for t in range(NT) for nm in ("k", "v")]
    cin_m = {"k": ck, "v": cv}
    sin_m = {"k": sk, "v": sv}
    out_m = {"k": ok, "v": ov}

    with TileContext(nc) as tc:
        with tc.tile_pool(name="row", bufs=1) as rowpool, \
             tc.tile_pool(name="cp", bufs=6) as cpool, \
             tc.tile_pool(name="sp", bufs=6) as spool:
            # --- small loads first on the scalar ring ---
            rt = {}
            for nm, nt_in in (("k", nk), ("v", nv)):
                rt[nm] = rowpool.tile([HSH * B, D], f32, tag=f"rt_{nm}",
                                      name=f"rt_{nm}")
                nc.scalar.dma_start(rt[nm][:], nt_in[:])
            st2 = {}
            for nm in ("k", "v"):
                stf = spool.tile([128, NT * NBLK], f32, tag="st",
                                 name=f"st_{nm}")
                nc.scalar.dma_start(stf[:], sin_m[nm][:])
                st2[nm] = spool.tile([128, NT * NBLK], f16, tag="st2",
                                     name=f"st2_{nm}")
                nc.vector.tensor_scalar(st2[nm][:], stf[:], C_DEQ, None,
                                        op.mult)

            # --- all bulk loads, alternating rings ---
            ct = {}
            for i, (t, nm) in enumerate(tiles):
                ct[t, nm] = cpool.tile([128, F], f16, tag="ct",
                                       name=f"ct_{t}_{nm}")
                nc.sync.dma_start(ct[t, nm][:], cin_m[nm][t])

            # --- dequantized replacement row for K and V (tiny) ---
            # f32 math matches the reference: `127.5/scale` lowers to
            # `127.5 * reciprocal(scale)`; rint() is the magic-constant
            # trick.  Only the final product is rounded to fp16.
            drow = {}
            for nm in ("k", "v"):
                sig = rowpool.tile([HSH * B, 1], f32, tag=f"sig_{nm}")
                nc.vector.tensor_reduce(sig[:], rt[nm][:],
                                        axis=mybir.AxisListType.X,
                                        op=op.max, apply_absolute_value=True)
                rc = rowpool.tile([HSH * B, 1], f32, tag=f"rc_{nm}")
                nc.vector.reciprocal(rc[:], sig[:])
                rr = rowpool.tile([HSH * B, 1], f32, tag=f"rr_{nm}")
                nc.vector.tensor_scalar(rr[:], rc[:], MAX_INT8, None, op.mult)
                tt = rowpool.tile([HSH * B, D], f32, tag=f"tt_{nm}")
                nc.vector.tensor_scalar(tt[:], rt[nm][:], rr[:], None, op.mult)
                qt = rowpool.tile([HSH * B, D], f32, tag=f"qt_{nm}")
                nc.vector.tensor_scalar(qt[:], tt[:], MAGIC, None, op.add)
                s2 = rowpool.tile([HSH * B, 1], f32, tag=f"s2_{nm}")
                nc.vector.tensor_scalar(s2[:], sig[:], C_DEQ, None, op.mult)
                dr = rowpool.tile([HSH * B, D], f16, tag=f"dr_{nm}")
                nc.vector.tensor_scalar(dr[:], qt[:], MAGIC, s2[:],
                                        op.subtract, op.mult)
                drow[nm] = dr

            # --- bulk dequantize: out = cache * (scale / 127.5) ---
            for i, (t, nm) in enumerate(tiles):
                c = ct[t, nm]
                ct3 = c[:].rearrange("p (j f) -> p j f", f=D)
                stb = (st2[nm][:, t * NBLK:(t + 1) * NBLK]
                       .unsqueeze(2).broadcast_to((128, NBLK, D)))
                nc.vector.tensor_tensor(ct3, ct3, stb, op.mult)
                if t == t_star:
                    nc.scalar.dma_start(
                        c[p_star:p_star + 1, f_star:f_star + ROWB],
                        drow[nm][:])
                nc.scalar.dma_start(out_m[nm][t], c[:])
    nc.compile()
    return nc


def _prog(s: int):
    if s not in _PROG_CACHE:
        _PROG_CACHE[s] = _build_program(s)
    return _PROG_CACHE[s]


def kernel(key, value, cached_ar_key, cached_ar_value,
           cached_ar_key_scale, cached_ar_value_scale, ar_cache_index):
    global LAST_RESULT
    from concourse.bass_utils import run_bass_kernel_spmd

    key = np.asarray(key, dtype=np.float32)
    value = np.asarray(value, dtype=np.float32)
    cached_ar_key = np.asarray(cached_ar_key, dtype=np.float32)
    cached_ar_value = np.asarray(cached_ar_value, dtype=np.float32)
    cached_ar_key_scale = np.asarray(cached_ar_key_scale, dtype=np.float32)
    cached_ar_value_scale = np.asarray(cached_ar_value_scale, dtype=np.float32)
    s = int(ar_cache_index)

    nc = _prog(s)

    key_t = np.ascontiguousarray(key[:, 0].transpose(1, 0, 2))      # (H,B,D)
    val_t = np.ascontiguousarray(value[:, 0].transpose(1, 0, 2))

    # The stored cache is int8-valued (|q| < 2048), so the fp16 staging
    # cast below is lossless re-encoding.
    in_maps = []
    for i in range(NCORES):
        h0 = i * HSH
        hs = slice(h0, h0 + HSH)
        in_maps.append({
            "ck": cached_ar_key[:, hs].astype(np.float16).reshape(NT, 128, F),
            "cv": cached_ar_value[:, hs].astype(np.float16).reshape(NT, 128, F),
            "sk": np.ascontiguousarray(
                cached_ar_key_scale[:, hs].reshape(NT, 128, NBLK)
                .transpose(1, 0, 2)).reshape(128, NT * NBLK),
            "sv": np.ascontiguousarray(
                cached_ar_value_scale[:, hs].reshape(NT, 128, NBLK)
                .transpose(1, 0, 2)).reshape(128, NT * NBLK),
            "nk": key_t[hs].reshape(HSH * B, D).copy(),
            "nv": val_t[hs].reshape(HSH * B, D).copy(),
        })

    res = run_bass_kernel_spmd(nc, in_maps, list(range(NCORES)), trace=TRACE)
    LAST_RESULT = res

    k_out = np.empty((S_AR, H, B, D), np.float32)
    v_out = np.empty((S_AR, H, B, D), np.float32)
    for i, r in enumerate(res.results):
        h0 = i * HSH
        k_out[:, h0:h0 + HSH] = r["ok"].reshape(S_AR, HSH, B, D).astype(np.float32)
        v_out[:, h0:h0 + HSH] = r["ov"].reshape(S_AR, HSH, B, D).astype(np.float32)
    return k_out, v_out
